# revision 15
# baseline (speedup 1.0000x reference)
"""Block-causal multi-head self-attention for TRN2, sharded over 8 NeuronCores.

Problem (hardcoded): B=2, T=2048 (512 frames x 4 animals), C=512, H=8 heads,
D=64. Block-causal mask = kron(tril(frames), ones(4,4)); key mask is all-ones
per the input spec (a numpy fallback handles the general case).

Sharding: core i handles batch b = i//4 and heads (2*(i%4), 2*(i%4)+1).
Wq/Wk/Wv are column-sharded (128 cols per core), Wp row-sharded (128 rows per
core). Each core emits a bf16 partial [T, C] output projection; the host sums
the 4 partials per batch (fp32) and adds bp.

On-core algorithm (bf16 matmul data, fp32 PSUM accumulation), fully
software-pipelined: per 512-token chunk t5, the q/k/v projections and V-block
transposes are emitted, then the attention groups for query chunks 2*t5 and
2*t5+1 flow through an S -> exp -> P@V pipeline with 2-group lookahead so the
PE never waits on the scalar-engine exp. Mask handled by a rank-64 indicator
matmul on diagonal blocks (-1e9 offside). The softmax denominator l rides as
row 64 of the accumulated O^T (ones column in V_aug); division by l happens
after the output projection, per 128-row tile, split between ACT and DVE.
"""

import math

import numpy as np
import ml_dtypes

import concourse.bass as bass
import concourse.bacc as bacc
import concourse.tile as tile
from concourse import mybir
from concourse.bass_utils import run_bass_kernel_spmd

B, T, C, H, D = 2, 2048, 512, 8, 64
NF, NA = 512, 4
NCORES = 8
HPC = 2            # heads per core
CPB = 4            # cores per batch
SCALE = 1.0 / math.sqrt(D)
NEG = -1.0e9
IC = 256           # query-chunk width
NQC = T // IC      # 8
GRP = 4            # j-blocks (128 keys each) per exp group

F32 = mybir.dt.float32
BF16 = mybir.dt.bfloat16
NPBF16 = ml_dtypes.bfloat16

USE_BIAS = False     # set by kernel() when any bias is nonzero
DBG_NOMASK = False


def _emit(ctx, tc, out_d, in_d):
    nc = tc.nc
    WDT = BF16
    ts = bass.ts
    Exp = mybir.ActivationFunctionType.Exp
    mult, add = mybir.AluOpType.mult, mybir.AluOpType.add

    const = ctx.enter_context(tc.tile_pool(name="const", bufs=1))
    big = ctx.enter_context(tc.tile_pool(name="big", bufs=1))

    # --- SBUF tiles -------------------------------------------------------
    sel = const.tile([65, 1], WDT)
    ident = const.tile([128, 128], WDT)
    ones_col = const.tile([128, 1], WDT)
    w_sb = {nm: const.tile([128, 512], WDT, tag=f"w_{nm}", name=f"w_{nm}")
            for nm in ("wq", "wk", "wv")}
    wps = [const.tile([64, 512], WDT, tag=f"wp{h}", name=f"wp{h}")
           for h in range(HPC)]
    if USE_BIAS:
        b3 = const.tile([1, 384], WDT, tag="b3")
        ones_row = const.tile([1, T], WDT, tag="ones")

    xT = big.tile([128, 4 * T], WDT, tag="xT")
    qA = [big.tile([128, T], WDT, tag=f"qA{h}", name=f"qA{h}") for h in range(HPC)]
    kA = [big.tile([128, T], WDT, tag=f"kA{h}", name=f"kA{h}") for h in range(HPC)]
    vT = big.tile([128, T], WDT, tag="vT")
    Vb = big.tile([128, 16 * 130], WDT, tag="Vb")
    OTs = [big.tile([65, T], WDT, tag=f"OT{h}", name=f"OT{h}") for h in range(HPC)]
    rl = big.tile([128, 2 * 16], F32, tag="rl")

    # --- input DMAs, split across the two HWDGE rings ---------------------
    # sync ring: wq, xt(t5=0), wk, wv, wp, xt(t5=2) in need order
    nc.sync.dma_start(out=w_sb["wq"][:], in_=in_d["wq"][:])
    for cb in range(4):
        nc.sync.dma_start(
            out=xT[:, cb * T : cb * T + 512], in_=in_d["xt"][ts(cb, 128), 0:512]
        )
    nc.sync.dma_start(out=w_sb["wk"][:], in_=in_d["wk"][:])
    nc.sync.dma_start(out=w_sb["wv"][:], in_=in_d["wv"][:])
    nc.sync.dma_start(out=wps[0][:], in_=in_d["wp"][0:64, :])
    nc.sync.dma_start(out=wps[1][:], in_=in_d["wp"][64:128, :])
    for cb in range(4):
        nc.sync.dma_start(
            out=xT[:, cb * T + 1024 : cb * T + 1536],
            in_=in_d["xt"][ts(cb, 128), 1024:1536],
        )
    # scalar ring: xt(t5=1), masks + consts (needed ~15us in), xt(t5=3)
    for cb in range(4):
        nc.scalar.dma_start(
            out=xT[:, cb * T + 512 : cb * T + 1024],
            in_=in_d["xt"][ts(cb, 128), 512:1024],
        )
    nc.scalar.dma_start(out=kA[0][64:128, :], in_=in_d["mask_k"][:])
    nc.scalar.dma_start(out=kA[1][0:64, :], in_=in_d["mask_k"][:])
    nc.scalar.dma_start(out=ident[:], in_=in_d["ident"][:])
    nc.scalar.dma_start(out=qA[0][64:128, :], in_=in_d["mask_q"][:])
    nc.scalar.dma_start(out=qA[1][0:64, :], in_=in_d["mask_q"][:])
    nc.scalar.dma_start(out=sel[:], in_=in_d["sel65"][:])
    nc.scalar.dma_start(out=ones_col[:], in_=in_d["ones_col"][:])
    if USE_BIAS:
        nc.scalar.dma_start(out=b3[:], in_=in_d["b3"][:])
        nc.scalar.dma_start(out=ones_row[:], in_=in_d["ones_row"][:])
    for cb in range(4):
        nc.scalar.dma_start(
            out=xT[:, cb * T + 1536 : cb * T + 2048],
            in_=in_d["xt"][ts(cb, 128), 1536:2048],
        )

    # Vb ones columns (constant): cols jb*130+64 and jb*130+129.
    for jb in range(16):
        nc.gpsimd.tensor_copy(Vb[:, jb * 130 + 64 : jb * 130 + 65], ones_col[:])
        nc.gpsimd.tensor_copy(Vb[:, jb * 130 + 129 : jb * 130 + 130], ones_col[:])

    # --- attention group list --------------------------------------------
    flat = []
    for qc in range(NQC):
        nbj = 2 * (qc + 1)
        for h in range(HPC):
            for g0 in range(0, nbj, GRP):
                flat.append((qc, h, g0, min(GRP, nbj - g0), nbj))
    n = len(flat)
    gstart = {}  # t5 -> first group index of qc == 2*t5
    for i, (qc, hh, g0, ng, nbj) in enumerate(flat):
        gstart.setdefault(qc // 2, i)
    gend = {t5: gstart.get(t5 + 1, n) for t5 in range(4)}

    pss = ctx.enter_context(tc.tile_pool(name="pss", bufs=2, space="PSUM"))
    psot = ctx.enter_context(tc.tile_pool(name="psot", bufs=1, space="PSUM"))
    prl = ctx.enter_context(tc.tile_pool(name="prl", bufs=2, space="PSUM"))
    aux = ctx.enter_context(tc.tile_pool(name="aux", bufs=1, space="PSUM"))
    ptp = ctx.enter_context(tc.tile_pool(name="ptp", bufs=3))
    comb = ctx.enter_context(tc.tile_pool(name="comb", bufs=2))

    ps_t = {}
    ot_t = {}
    s_ptr = [0]

    def emit_s(i):
        qc, h, g0, ng, nbj = flat[i]
        ps = pss.tile([128, GRP * IC], F32, tag="ps", name=f"ps{i}")
        ps_t[i] = ps
        hs = slice(0, 64) if h == 0 else slice(64, 128)
        for k in range(ng):
            jb = g0 + k
            sl = ps[:, ts(k, IC)]
            if jb >= 2 * qc and not DBG_NOMASK:
                nc.tensor.matmul(
                    sl, kA[h][:, ts(jb, 128)], qA[h][:, ts(qc, IC)],
                    start=True, stop=True,
                )
            else:
                nc.tensor.matmul(
                    sl, kA[h][hs, ts(jb, 128)], qA[h][hs, ts(qc, IC)],
                    start=True, stop=True,
                )

    def pump_s(upto):
        while s_ptr[0] < min(upto, n):
            emit_s(s_ptr[0])
            s_ptr[0] += 1

    def emit_proj(nm, dst, t5):
        ps = aux.tile([128, 512], F32, tag="aux", name=f"pj_{nm}{t5}")
        for cb in range(4):
            nc.tensor.matmul(
                ps[:],
                w_sb[nm][:, ts(cb, 128)],
                xT[:, cb * T + t5 * 512 : cb * T + (t5 + 1) * 512],
                start=(cb == 0),
                stop=(False if USE_BIAS else cb == 3),
            )
        if USE_BIAS:
            boff = {"wq": 0, "wk": 128, "wv": 256}[nm]
            nc.tensor.matmul(
                ps[:], b3[:, boff : boff + 128], ones_row[:, ts(t5, 512)],
                start=False, stop=True,
            )
        if nm == "wv":
            nc.vector.tensor_copy(dst[:, ts(t5, 512)], ps[:])
        else:
            nc.vector.tensor_copy(dst[0][0:64, ts(t5, 512)], ps[0:64, :])
            nc.vector.tensor_copy(dst[1][64:128, ts(t5, 512)], ps[64:128, :])

    def emit_vb(t5):
        for jb in range(4 * t5, 4 * t5 + 4):
            pv = aux.tile([128, 128], WDT, tag="aux", name=f"pv{jb}")
            nc.tensor.transpose(pv[:], vT[:, ts(jb, 128)], ident[:])
            nc.vector.tensor_copy(Vb[:, jb * 130 : jb * 130 + 64], pv[:, 0:64])
            nc.vector.tensor_copy(
                Vb[:, jb * 130 + 65 : jb * 130 + 129], pv[:, 64:128]
            )

    def emit_tail(qc):
        for t2 in (2 * qc, 2 * qc + 1):
            pr1 = prl.tile([128, 512], F32, tag="pr", name=f"pr1_{t2}")
            for h in range(HPC):
                nc.tensor.matmul(
                    pr1[:, h : h + 1], OTs[h][:, ts(t2, 128)], sel[:],
                    start=True, stop=True,
                )
            nc.vector.reciprocal(rl[:, 2 * t2 : 2 * t2 + 2], pr1[:, 0:2])
            pr0 = prl.tile([128, 512], F32, tag="pr", name=f"pr0_{t2}")
            nc.tensor.matmul(
                pr0[:], OTs[0][0:64, ts(t2, 128)], wps[0][:],
                start=True, stop=True,
            )
            nc.tensor.matmul(
                pr1[:], OTs[1][0:64, ts(t2, 128)], wps[1][:],
                start=True, stop=True,
            )
            tmp = comb.tile([128, 512], F32, tag="tmp")
            if t2 % 2 == 0:
                nc.scalar.mul(tmp[:], pr0[:], rl[:, 2 * t2 : 2 * t2 + 1])
            else:
                nc.vector.tensor_scalar_mul(
                    tmp[:], pr0[:], rl[:, 2 * t2 : 2 * t2 + 1]
                )
            ob = comb.tile([128, 512], WDT, tag="ob")
            nc.vector.scalar_tensor_tensor(
                ob[:], pr1[:], rl[:, 2 * t2 + 1 : 2 * t2 + 2], tmp[:], mult, add
            )
            nc.sync.dma_start(out=out_d[ts(t2, 128), :], in_=ob[:])

    # --- main pipeline ----------------------------------------------------
    for t5 in range(4):
        emit_proj("wq", qA, t5)
        emit_proj("wk", kA, t5)
        pump_s(gstart[t5] + 2)
        emit_proj("wv", vT, t5)
        emit_vb(t5)
        for i in range(gstart[t5], gend[t5]):
            qc, h, g0, ng, nbj = flat[i]
            ps = ps_t.pop(i)
            ptt = ptp.tile([128, GRP * IC], WDT)
            nc.scalar.activation(ptt[:, 0 : ng * IC], ps[:, 0 : ng * IC], Exp)
            if g0 == 0 and h == 0:
                ot_t[qc] = psot.tile([65, HPC * IC], F32, tag="ot", name=f"ot{qc}")
            ot = ot_t[qc]
            osl = ot[:, h * IC : (h + 1) * IC]
            for k in range(ng):
                jb = g0 + k
                vbase = jb * 130 + h * 65
                nc.tensor.matmul(
                    osl, Vb[:, vbase : vbase + 65], ptt[:, ts(k, IC)],
                    start=(jb == 0), stop=(jb == nbj - 1),
                )
            pump_s(min(i + 3, gend[t5]))
            if g0 + ng == nbj:
                nc.vector.tensor_copy(OTs[h][:, ts(qc, IC)], osl)
                if h == HPC - 1:
                    emit_tail(qc)


_PROGRAM_CACHE = {}
TRACE = False
_LAST = {}


def _build_program():
    key = ("prog", GRP, USE_BIAS, DBG_NOMASK)
    if key in _PROGRAM_CACHE:
        return _PROGRAM_CACHE[key]
    from contextlib import ExitStack

    nc = bacc.Bacc(trn_type="TRN2", target_bir_lowering=False, debug=False,
                   num_devices=NCORES)
    WDT = BF16
    in_d = {
        "xt": nc.dram_tensor("xt", [C, T], WDT, kind="ExternalInput").ap(),
        "wq": nc.dram_tensor("wq", [128, 512], WDT, kind="ExternalInput").ap(),
        "wk": nc.dram_tensor("wk", [128, 512], WDT, kind="ExternalInput").ap(),
        "wv": nc.dram_tensor("wv", [128, 512], WDT, kind="ExternalInput").ap(),
        "wp": nc.dram_tensor("wp", [128, C], WDT, kind="ExternalInput").ap(),
        "ident": nc.dram_tensor("ident", [128, 128], WDT, kind="ExternalInput").ap(),
        "mask_k": nc.dram_tensor("mask_k", [64, T], WDT, kind="ExternalInput").ap(),
        "mask_q": nc.dram_tensor("mask_q", [64, T], WDT, kind="ExternalInput").ap(),
        "sel65": nc.dram_tensor("sel65", [65, 1], WDT, kind="ExternalInput").ap(),
        "ones_col": nc.dram_tensor("ones_col", [128, 1], WDT,
                                   kind="ExternalInput").ap(),
    }
    if USE_BIAS:
        in_d["b3"] = nc.dram_tensor("b3", [1, 384], WDT, kind="ExternalInput").ap()
        in_d["ones_row"] = nc.dram_tensor(
            "ones_row", [1, T], WDT, kind="ExternalInput").ap()
    out_d = nc.dram_tensor("out", [T, C], BF16, kind="ExternalOutput").ap()
    with tile.TileContext(nc) as tc:
        with ExitStack() as ctx:
            _emit(ctx, tc, out_d, in_d)
    nc.compile()
    _PROGRAM_CACHE[key] = nc
    return nc


def _consts():
    f = np.float32
    rr = np.arange(64)
    jj = np.arange(T)
    # mask_k[r, j] = 1 where r == 32*(jb%2) + (j%128)//4 (jb = j//128)
    mk = (rr[:, None] == 32 * ((jj[None, :] // 128) % 2) + (jj[None, :] % 128) // NA)
    mask_k = mk.astype(f)
    # mask_q[r, i] = NEG where (i%256)//4 < r
    mask_q = np.where((jj[None, :] % IC) // NA < rr[:, None], f(NEG), f(0.0)).astype(f)
    sel = np.zeros((65, 1), dtype=f)
    sel[64, 0] = 1.0
    ident = np.eye(128, dtype=f)
    return mask_k, mask_q, sel, ident


def _sbuf_w(w):
    # DRAM [512, 128] -> SBUF-layout [128, 512]: w_sb[p, cb*128+f] = W[cb*128+p, f]
    return np.ascontiguousarray(
        w.reshape(4, 128, 128).transpose(1, 0, 2).reshape(128, 512)
    )


def _numpy_reference(x, mask, Wq, bq, Wk, bk, Wv, bv, Wp, bp):
    b, t, c = x.shape
    h, d = H, c // H
    scale = 1.0 / math.sqrt(d)
    tril = np.tril(np.ones((NF, NF), dtype=np.float32))
    block = np.kron(tril, np.ones((NA, NA), dtype=np.float32))

    def heads(w, bias):
        return (x @ w + bias).reshape(b, t, h, d).transpose(0, 2, 1, 3)

    q, k, v = heads(Wq, bq), heads(Wk, bk), heads(Wv, bv)
    att = np.einsum("bhqd,bhkd->bhqk", q, k) * scale
    allowed = block[None, None] * mask[:, None, None, :].astype(np.float32)
    att = np.where(allowed == 0, -np.inf, att)
    att = att - att.max(axis=-1, keepdims=True)
    att = np.exp(att)
    att = att / att.sum(axis=-1, keepdims=True)
    y = np.einsum("bhqk,bhkd->bhqd", att, v)
    y = y.transpose(0, 2, 1, 3).reshape(b, t, c)
    return (y @ Wp + bp).astype(np.float32)


def kernel(**inputs):
    global USE_BIAS
    x = np.asarray(inputs["x"], dtype=np.float32)
    mask = np.asarray(inputs["mask"])
    Wq = np.asarray(inputs["Wq"], dtype=np.float32)
    bq = np.asarray(inputs["bq"], dtype=np.float32)
    Wk = np.asarray(inputs["Wk"], dtype=np.float32)
    bk = np.asarray(inputs["bk"], dtype=np.float32)
    Wv = np.asarray(inputs["Wv"], dtype=np.float32)
    bv = np.asarray(inputs["bv"], dtype=np.float32)
    Wp = np.asarray(inputs["Wp"], dtype=np.float32)
    bp = np.asarray(inputs["bp"], dtype=np.float32)

    if not np.all(np.asarray(mask) == 1):
        return _numpy_reference(x, mask, Wq, bq, Wk, bk, Wv, bv, Wp, bp)

    USE_BIAS = bool(np.any(bq) or np.any(bk) or np.any(bv))
    nc = _build_program()
    mask_k, mask_q, sel, ident = _consts()
    bf = NPBF16
    in_maps = []
    for core in range(NCORES):
        b = core // CPB
        hp = core % CPB
        cs = slice(hp * 128, (hp + 1) * 128)
        im = {
            "xt": np.ascontiguousarray(x[b].T).astype(bf),
            "wq": (_sbuf_w(np.ascontiguousarray(Wq[:, cs]))
                   * np.float32(SCALE)).astype(bf),
            "wk": _sbuf_w(np.ascontiguousarray(Wk[:, cs])).astype(bf),
            "wv": _sbuf_w(np.ascontiguousarray(Wv[:, cs])).astype(bf),
            "wp": np.ascontiguousarray(Wp[cs, :]).astype(bf),
            "ident": ident.astype(bf),
            "mask_k": mask_k.astype(bf),
            "mask_q": mask_q.astype(bf),
            "sel65": sel.astype(bf),
            "ones_col": np.ones((128, 1), dtype=bf),
        }
        if USE_BIAS:
            b3 = np.concatenate([bq[cs] * np.float32(SCALE), bk[cs], bv[cs]])
            im["b3"] = b3.reshape(1, 384).astype(bf)
            im["ones_row"] = np.ones((1, T), dtype=bf)
        in_maps.append(im)
    rr = run_bass_kernel_spmd(
        nc, in_maps, list(range(NCORES)), trace=TRACE,
        tmpdir=_LAST.get("tmpdir"),
    )
    _LAST["results"] = rr
    res = rr.results
    out = np.zeros((B, T, C), dtype=np.float32)
    for b in range(B):
        acc = res[b * CPB]["out"].astype(np.float32)
        for j in range(1, CPB):
            acc = acc + res[b * CPB + j]["out"].astype(np.float32)
        out[b] = acc + bp[None, :]
    return out


# revision 17
# speedup vs baseline: 1.0402x; 1.0402x over previous
"""Block-causal multi-head self-attention for TRN2, sharded over 8 NeuronCores.

Problem (hardcoded): B=2, T=2048 (512 frames x 4 animals), C=512, H=8 heads,
D=64. Block-causal mask = kron(tril(frames), ones(4,4)); key mask is all-ones
per the input spec (a numpy fallback handles the general case).

Sharding: core i handles batch b = i//4 and heads (2*(i%4), 2*(i%4)+1).
Wq/Wk/Wv are column-sharded (128 cols per core), Wp row-sharded (128 rows per
core). Each core emits a bf16 partial [T, C] output projection; the host sums
the 4 partials per batch (fp32) and adds bp.

On-core algorithm (bf16 matmul data, fp32 PSUM accumulation), fully
software-pipelined: per 512-token chunk t5, the q/k/v projections and V-block
transposes are emitted, then the attention groups for query chunks 2*t5 and
2*t5+1 flow through an S -> exp -> P@V pipeline with 2-group lookahead so the
PE never waits on the scalar-engine exp. Mask handled by a rank-64 indicator
matmul on diagonal blocks (-1e9 offside). The softmax denominator l rides as
row 64 of the accumulated O^T (ones column in V_aug); division by l happens
after the output projection, per 128-row tile, split between ACT and DVE.
"""

import math

import numpy as np
import ml_dtypes

import concourse.bass as bass
import concourse.bacc as bacc
import concourse.tile as tile
from concourse import mybir
from concourse.bass_utils import run_bass_kernel_spmd

B, T, C, H, D = 2, 2048, 512, 8, 64
NF, NA = 512, 4
NCORES = 8
HPC = 2            # heads per core
CPB = 4            # cores per batch
SCALE = 1.0 / math.sqrt(D)
NEG = -1.0e9
IC = 256           # query-chunk width
NQC = T // IC      # 8
GRP = 4            # j-blocks (128 keys each) per exp group

F32 = mybir.dt.float32
BF16 = mybir.dt.bfloat16
NPBF16 = ml_dtypes.bfloat16

USE_BIAS = False     # set by kernel() when any bias is nonzero
DBG_NOMASK = False


def _emit(ctx, tc, out_d, in_d):
    nc = tc.nc
    WDT = BF16
    ts = bass.ts
    Exp = mybir.ActivationFunctionType.Exp
    mult, add = mybir.AluOpType.mult, mybir.AluOpType.add

    const = ctx.enter_context(tc.tile_pool(name="const", bufs=1))
    big = ctx.enter_context(tc.tile_pool(name="big", bufs=1))

    # --- SBUF tiles -------------------------------------------------------
    sel = const.tile([65, 1], WDT)
    ident = const.tile([128, 128], WDT)
    ones_col = const.tile([128, 1], WDT)
    w_sb = {nm: const.tile([128, 512], WDT, tag=f"w_{nm}", name=f"w_{nm}")
            for nm in ("wq", "wk", "wv")}
    wps = [const.tile([64, 512], WDT, tag=f"wp{h}", name=f"wp{h}")
           for h in range(HPC)]
    if USE_BIAS:
        b3 = const.tile([1, 384], WDT, tag="b3")
        ones_row = const.tile([1, T], WDT, tag="ones")

    xT = big.tile([128, 4 * T], WDT, tag="xT")
    qA = [big.tile([128, T], WDT, tag=f"qA{h}", name=f"qA{h}") for h in range(HPC)]
    kA = [big.tile([128, T], WDT, tag=f"kA{h}", name=f"kA{h}") for h in range(HPC)]
    vT = big.tile([128, T], WDT, tag="vT")
    Vb = big.tile([128, 16 * 130], WDT, tag="Vb")
    OTs = [big.tile([65, T], WDT, tag=f"OT{h}", name=f"OT{h}") for h in range(HPC)]
    rl = big.tile([128, 2 * 16], F32, tag="rl")

    # --- input DMAs, split across the two HWDGE rings ---------------------
    # sync ring: wq, xt(t5=0), wk, wv, wp, xt(t5=2) in need order
    nc.sync.dma_start(out=w_sb["wq"][:], in_=in_d["wq"][:])
    for cb in range(4):
        nc.sync.dma_start(
            out=xT[:, cb * T : cb * T + 512], in_=in_d["xt"][ts(cb, 128), 0:512]
        )
    nc.sync.dma_start(out=w_sb["wk"][:], in_=in_d["wk"][:])
    nc.sync.dma_start(out=w_sb["wv"][:], in_=in_d["wv"][:])
    nc.sync.dma_start(out=wps[0][:], in_=in_d["wp"][0:64, :])
    nc.sync.dma_start(out=wps[1][:], in_=in_d["wp"][64:128, :])
    for cb in range(4):
        nc.sync.dma_start(
            out=xT[:, cb * T + 1024 : cb * T + 1536],
            in_=in_d["xt"][ts(cb, 128), 1024:1536],
        )
    # scalar ring: xt(t5=1), masks + consts (needed ~15us in), xt(t5=3)
    for cb in range(4):
        nc.scalar.dma_start(
            out=xT[:, cb * T + 512 : cb * T + 1024],
            in_=in_d["xt"][ts(cb, 128), 512:1024],
        )
    nc.scalar.dma_start(out=kA[0][64:128, :], in_=in_d["mask_k"][:])
    nc.scalar.dma_start(out=kA[1][0:64, :], in_=in_d["mask_k"][:])
    nc.scalar.dma_start(out=ident[:], in_=in_d["ident"][:])
    nc.scalar.dma_start(out=qA[0][64:128, :], in_=in_d["mask_q"][:])
    nc.scalar.dma_start(out=qA[1][0:64, :], in_=in_d["mask_q"][:])
    nc.scalar.dma_start(out=sel[:], in_=in_d["sel65"][:])
    nc.scalar.dma_start(out=ones_col[:], in_=in_d["ones_col"][:])
    if USE_BIAS:
        nc.scalar.dma_start(out=b3[:], in_=in_d["b3"][:])
        nc.scalar.dma_start(out=ones_row[:], in_=in_d["ones_row"][:])
    for cb in range(4):
        nc.scalar.dma_start(
            out=xT[:, cb * T + 1536 : cb * T + 2048],
            in_=in_d["xt"][ts(cb, 128), 1536:2048],
        )

    # Vb ones columns (constant): cols jb*130+64 and jb*130+129.
    for jb in range(16):
        nc.gpsimd.tensor_copy(Vb[:, jb * 130 + 64 : jb * 130 + 65], ones_col[:])
        nc.gpsimd.tensor_copy(Vb[:, jb * 130 + 129 : jb * 130 + 130], ones_col[:])

    # --- attention stage machinery ---------------------------------------
    # Per t5-pair (query chunks qcE=2*t5, qcO=2*t5+1):
    #   Pair stages: off-diagonal key blocks jb < 4*t5 are scored for BOTH
    #     query chunks in one 512-wide matmul; exp lands in a persistent
    #     SBUF tile pttP, consumed now for qcE and later for qcO.
    #   DiagE: qcE's two diagonal blocks (K=128 with mask rows), then tail.
    #   FreshO: qcO's four remaining blocks + deferred paired P@V, then tail.
    pss = ctx.enter_context(tc.tile_pool(name="pss", bufs=2, space="PSUM"))
    psot = ctx.enter_context(tc.tile_pool(name="psot", bufs=1, space="PSUM"))
    prl = ctx.enter_context(tc.tile_pool(name="prl", bufs=2, space="PSUM"))
    aux = ctx.enter_context(tc.tile_pool(name="aux", bufs=1, space="PSUM"))
    ptp = ctx.enter_context(tc.tile_pool(name="ptp", bufs=3))
    pttp = ctx.enter_context(tc.tile_pool(name="pttp", bufs=1))
    comb = ctx.enter_context(tc.tile_pool(name="comb", bufs=2))

    state = {}

    def hsl(h):
        return slice(0, 64) if h == 0 else slice(64, 128)

    def pv_mm(ot, h, jb, mov, start, stop):
        vbase = jb * 130 + h * 65
        nc.tensor.matmul(
            ot[:, h * IC : (h + 1) * IC], Vb[:, vbase : vbase + 65], mov,
            start=start, stop=stop,
        )

    def get_ot(key):
        if key not in state:
            state[key] = psot.tile([65, HPC * IC], F32, tag="ot", name=f"ot{key}")
        return state[key]

    def emit_proj(nm, dst, t5):
        ps = aux.tile([128, 512], F32, tag="aux", name=f"pj_{nm}{t5}")
        for cb in range(4):
            nc.tensor.matmul(
                ps[:],
                w_sb[nm][:, ts(cb, 128)],
                xT[:, cb * T + t5 * 512 : cb * T + (t5 + 1) * 512],
                start=(cb == 0),
                stop=(False if USE_BIAS else cb == 3),
            )
        if USE_BIAS:
            boff = {"wq": 0, "wk": 128, "wv": 256}[nm]
            nc.tensor.matmul(
                ps[:], b3[:, boff : boff + 128], ones_row[:, ts(t5, 512)],
                start=False, stop=True,
            )
        if nm == "wv":
            nc.vector.tensor_copy(dst[:, ts(t5, 512)], ps[:])
        else:
            nc.vector.tensor_copy(dst[0][0:64, ts(t5, 512)], ps[0:64, :])
            nc.vector.tensor_copy(dst[1][64:128, ts(t5, 512)], ps[64:128, :])

    def emit_vb(t5):
        for jb in range(4 * t5, 4 * t5 + 4):
            pv = aux.tile([128, 128], WDT, tag="aux", name=f"pv{jb}")
            nc.tensor.transpose(pv[:], vT[:, ts(jb, 128)], ident[:])
            nc.vector.tensor_copy(Vb[:, jb * 130 : jb * 130 + 64], pv[:, 0:64])
            nc.vector.tensor_copy(
                Vb[:, jb * 130 + 65 : jb * 130 + 129], pv[:, 64:128]
            )

    def emit_tail(qc):
        for t2 in (2 * qc, 2 * qc + 1):
            pr1 = prl.tile([128, 512], F32, tag="pr", name=f"pr1_{t2}")
            for h in range(HPC):
                nc.tensor.matmul(
                    pr1[:, h : h + 1], OTs[h][:, ts(t2, 128)], sel[:],
                    start=True, stop=True,
                )
            nc.vector.reciprocal(rl[:, 2 * t2 : 2 * t2 + 2], pr1[:, 0:2])
            pr0 = prl.tile([128, 512], F32, tag="pr", name=f"pr0_{t2}")
            nc.tensor.matmul(
                pr0[:], OTs[0][0:64, ts(t2, 128)], wps[0][:],
                start=True, stop=True,
            )
            nc.tensor.matmul(
                pr1[:], OTs[1][0:64, ts(t2, 128)], wps[1][:],
                start=True, stop=True,
            )
            tmp = comb.tile([128, 512], F32, tag="tmp")
            nc.vector.tensor_scalar_mul(tmp[:], pr0[:], rl[:, 2 * t2 : 2 * t2 + 1])
            ob = comb.tile([128, 512], WDT, tag="ob")
            nc.vector.scalar_tensor_tensor(
                ob[:], pr1[:], rl[:, 2 * t2 + 1 : 2 * t2 + 2], tmp[:], mult, add
            )
            nc.sync.dma_start(out=out_d[ts(t2, 128), :], in_=ob[:])

    class Pair:
        def __init__(self, t5, h, c0):
            self.t5, self.h, self.c0 = t5, h, c0

        def s_phase(self, idx):
            t5, h, c0 = self.t5, self.h, self.c0
            ps = pss.tile([128, 1024], F32, tag="ps", name=f"ps{idx}")
            self.ps = ps
            for k in range(2):
                jb = c0 + k
                nc.tensor.matmul(
                    ps[:, ts(k, 512)],
                    kA[h][hsl(h), ts(jb, 128)],
                    qA[h][hsl(h), ts(t5, 512)],
                    start=True, stop=True,
                )

        def c_phase(self):
            t5, h, c0 = self.t5, self.h, self.c0
            pkey = ("pttp", t5, h)
            if pkey not in state:
                state[pkey] = pttp.tile(
                    [128, 12 * 512], WDT, tag=f"pttP{h}", name=f"pttP{h}_{t5}"
                )
            pt = state[pkey]
            nc.scalar.activation(pt[:, c0 * 512 : (c0 + 2) * 512], self.ps[:], Exp)
            ot = get_ot(("ot", t5, "e"))
            for k in range(2):
                jb = c0 + k
                pv_mm(ot, h, jb, pt[:, jb * 512 : jb * 512 + IC],
                      start=(jb == 0), stop=False)

    class DiagE:
        def __init__(self, t5, h):
            self.t5, self.h = t5, h

        def s_phase(self, idx):
            t5, h = self.t5, self.h
            ps = pss.tile([128, 1024], F32, tag="ps", name=f"ps{idx}")
            self.ps = ps
            for k in range(2):
                jb = 4 * t5 + k
                nc.tensor.matmul(
                    ps[:, ts(k, IC)],
                    kA[h][:, ts(jb, 128)],
                    qA[h][:, ts(2 * t5, IC)],
                    start=True, stop=True,
                )

        def c_phase(self):
            t5, h = self.t5, self.h
            ptt = ptp.tile([128, 1024], WDT, tag="ptt", name="ptt")
            nc.scalar.activation(ptt[:, 0:512], self.ps[:, 0:512], Exp)
            ot = get_ot(("ot", t5, "e"))
            for k in range(2):
                jb = 4 * t5 + k
                pv_mm(ot, h, jb, ptt[:, ts(k, IC)],
                      start=(jb == 0), stop=(k == 1))
            nc.vector.tensor_copy(
                OTs[h][:, ts(2 * t5, IC)], ot[:, h * IC : (h + 1) * IC]
            )
            if h == HPC - 1:
                emit_tail(2 * t5)

    class FreshO:
        def __init__(self, t5, h):
            self.t5, self.h = t5, h

        def s_phase(self, idx):
            t5, h = self.t5, self.h
            ps = pss.tile([128, 1024], F32, tag="ps", name=f"ps{idx}")
            self.ps = ps
            qsl = qA[h][:, 2 * t5 * IC + IC : 2 * t5 * IC + 2 * IC]
            qso = qA[h][hsl(h), 2 * t5 * IC + IC : 2 * t5 * IC + 2 * IC]
            for k in range(4):
                jb = 4 * t5 + k
                if k < 2:
                    nc.tensor.matmul(
                        self.ps[:, ts(k, IC)],
                        kA[h][hsl(h), ts(jb, 128)], qso,
                        start=True, stop=True,
                    )
                else:
                    nc.tensor.matmul(
                        self.ps[:, ts(k, IC)],
                        kA[h][:, ts(jb, 128)], qsl,
                        start=True, stop=True,
                    )

        def c_phase(self):
            t5, h = self.t5, self.h
            ptt = ptp.tile([128, 1024], WDT, tag="ptt", name="ptt")
            nc.scalar.activation(ptt[:], self.ps[:], Exp)
            ot = get_ot(("ot", t5, "o"))
            pt = state.get(("pttp", t5, h))
            for jb in range(4 * t5):
                pv_mm(ot, h, jb, pt[:, jb * 512 + IC : (jb + 1) * 512],
                      start=(jb == 0), stop=False)
            for k in range(4):
                jb = 4 * t5 + k
                pv_mm(ot, h, jb, ptt[:, ts(k, IC)],
                      start=(jb == 0), stop=(k == 3))
            nc.vector.tensor_copy(
                OTs[h][:, 2 * t5 * IC + IC : 2 * t5 * IC + 2 * IC],
                ot[:, h * IC : (h + 1) * IC],
            )
            if h == HPC - 1:
                emit_tail(2 * t5 + 1)

    # --- main pipeline ----------------------------------------------------
    sidx = [0]
    for t5 in range(4):
        stages = []
        for h in range(HPC):
            for c0 in range(0, 4 * t5, 2):
                stages.append(Pair(t5, h, c0))
            stages.append(DiagE(t5, h))
        for h in range(HPC):
            stages.append(FreshO(t5, h))

        emit_proj("wq", qA, t5)
        emit_proj("wk", kA, t5)
        stages[0].s_phase(sidx[0]); sidx[0] += 1
        if len(stages) > 1:
            stages[1].s_phase(sidx[0]); sidx[0] += 1
        emit_proj("wv", vT, t5)
        emit_vb(t5)
        for i, st in enumerate(stages):
            st.c_phase()
            if i + 2 < len(stages):
                stages[i + 2].s_phase(sidx[0]); sidx[0] += 1


_PROGRAM_CACHE = {}
TRACE = False
_LAST = {}


def _build_program():
    key = ("prog", GRP, USE_BIAS, DBG_NOMASK)
    if key in _PROGRAM_CACHE:
        return _PROGRAM_CACHE[key]
    from contextlib import ExitStack

    nc = bacc.Bacc(trn_type="TRN2", target_bir_lowering=False, debug=False,
                   num_devices=NCORES)
    WDT = BF16
    in_d = {
        "xt": nc.dram_tensor("xt", [C, T], WDT, kind="ExternalInput").ap(),
        "wq": nc.dram_tensor("wq", [128, 512], WDT, kind="ExternalInput").ap(),
        "wk": nc.dram_tensor("wk", [128, 512], WDT, kind="ExternalInput").ap(),
        "wv": nc.dram_tensor("wv", [128, 512], WDT, kind="ExternalInput").ap(),
        "wp": nc.dram_tensor("wp", [128, C], WDT, kind="ExternalInput").ap(),
        "ident": nc.dram_tensor("ident", [128, 128], WDT, kind="ExternalInput").ap(),
        "mask_k": nc.dram_tensor("mask_k", [64, T], WDT, kind="ExternalInput").ap(),
        "mask_q": nc.dram_tensor("mask_q", [64, T], WDT, kind="ExternalInput").ap(),
        "sel65": nc.dram_tensor("sel65", [65, 1], WDT, kind="ExternalInput").ap(),
        "ones_col": nc.dram_tensor("ones_col", [128, 1], WDT,
                                   kind="ExternalInput").ap(),
    }
    if USE_BIAS:
        in_d["b3"] = nc.dram_tensor("b3", [1, 384], WDT, kind="ExternalInput").ap()
        in_d["ones_row"] = nc.dram_tensor(
            "ones_row", [1, T], WDT, kind="ExternalInput").ap()
    out_d = nc.dram_tensor("out", [T, C], BF16, kind="ExternalOutput").ap()
    with tile.TileContext(nc) as tc:
        with ExitStack() as ctx:
            _emit(ctx, tc, out_d, in_d)
    nc.compile()
    _PROGRAM_CACHE[key] = nc
    return nc


def _consts():
    f = np.float32
    rr = np.arange(64)
    jj = np.arange(T)
    # mask_k[r, j] = 1 where r == 32*(jb%2) + (j%128)//4 (jb = j//128)
    mk = (rr[:, None] == 32 * ((jj[None, :] // 128) % 2) + (jj[None, :] % 128) // NA)
    mask_k = mk.astype(f)
    # mask_q[r, i] = NEG where (i%256)//4 < r
    mask_q = np.where((jj[None, :] % IC) // NA < rr[:, None], f(NEG), f(0.0)).astype(f)
    sel = np.zeros((65, 1), dtype=f)
    sel[64, 0] = 1.0
    ident = np.eye(128, dtype=f)
    return mask_k, mask_q, sel, ident


def _sbuf_w(w):
    # DRAM [512, 128] -> SBUF-layout [128, 512]: w_sb[p, cb*128+f] = W[cb*128+p, f]
    return np.ascontiguousarray(
        w.reshape(4, 128, 128).transpose(1, 0, 2).reshape(128, 512)
    )


def _numpy_reference(x, mask, Wq, bq, Wk, bk, Wv, bv, Wp, bp):
    b, t, c = x.shape
    h, d = H, c // H
    scale = 1.0 / math.sqrt(d)
    tril = np.tril(np.ones((NF, NF), dtype=np.float32))
    block = np.kron(tril, np.ones((NA, NA), dtype=np.float32))

    def heads(w, bias):
        return (x @ w + bias).reshape(b, t, h, d).transpose(0, 2, 1, 3)

    q, k, v = heads(Wq, bq), heads(Wk, bk), heads(Wv, bv)
    att = np.einsum("bhqd,bhkd->bhqk", q, k) * scale
    allowed = block[None, None] * mask[:, None, None, :].astype(np.float32)
    att = np.where(allowed == 0, -np.inf, att)
    att = att - att.max(axis=-1, keepdims=True)
    att = np.exp(att)
    att = att / att.sum(axis=-1, keepdims=True)
    y = np.einsum("bhqk,bhkd->bhqd", att, v)
    y = y.transpose(0, 2, 1, 3).reshape(b, t, c)
    return (y @ Wp + bp).astype(np.float32)


def kernel(**inputs):
    global USE_BIAS
    x = np.asarray(inputs["x"], dtype=np.float32)
    mask = np.asarray(inputs["mask"])
    Wq = np.asarray(inputs["Wq"], dtype=np.float32)
    bq = np.asarray(inputs["bq"], dtype=np.float32)
    Wk = np.asarray(inputs["Wk"], dtype=np.float32)
    bk = np.asarray(inputs["bk"], dtype=np.float32)
    Wv = np.asarray(inputs["Wv"], dtype=np.float32)
    bv = np.asarray(inputs["bv"], dtype=np.float32)
    Wp = np.asarray(inputs["Wp"], dtype=np.float32)
    bp = np.asarray(inputs["bp"], dtype=np.float32)

    if not np.all(np.asarray(mask) == 1):
        return _numpy_reference(x, mask, Wq, bq, Wk, bk, Wv, bv, Wp, bp)

    USE_BIAS = bool(np.any(bq) or np.any(bk) or np.any(bv))
    nc = _build_program()
    mask_k, mask_q, sel, ident = _consts()
    bf = NPBF16
    in_maps = []
    for core in range(NCORES):
        b = core // CPB
        hp = core % CPB
        cs = slice(hp * 128, (hp + 1) * 128)
        im = {
            "xt": np.ascontiguousarray(x[b].T).astype(bf),
            "wq": (_sbuf_w(np.ascontiguousarray(Wq[:, cs]))
                   * np.float32(SCALE)).astype(bf),
            "wk": _sbuf_w(np.ascontiguousarray(Wk[:, cs])).astype(bf),
            "wv": _sbuf_w(np.ascontiguousarray(Wv[:, cs])).astype(bf),
            "wp": np.ascontiguousarray(Wp[cs, :]).astype(bf),
            "ident": ident.astype(bf),
            "mask_k": mask_k.astype(bf),
            "mask_q": mask_q.astype(bf),
            "sel65": sel.astype(bf),
            "ones_col": np.ones((128, 1), dtype=bf),
        }
        if USE_BIAS:
            b3 = np.concatenate([bq[cs] * np.float32(SCALE), bk[cs], bv[cs]])
            im["b3"] = b3.reshape(1, 384).astype(bf)
            im["ones_row"] = np.ones((1, T), dtype=bf)
        in_maps.append(im)
    rr = run_bass_kernel_spmd(
        nc, in_maps, list(range(NCORES)), trace=TRACE,
        tmpdir=_LAST.get("tmpdir"),
    )
    _LAST["results"] = rr
    res = rr.results
    out = np.zeros((B, T, C), dtype=np.float32)
    for b in range(B):
        acc = res[b * CPB]["out"].astype(np.float32)
        for j in range(1, CPB):
            acc = acc + res[b * CPB + j]["out"].astype(np.float32)
        out[b] = acc + bp[None, :]
    return out


# revision 18
# speedup vs baseline: 1.0719x; 1.0305x over previous
"""Block-causal multi-head self-attention for TRN2, sharded over 8 NeuronCores.

Problem (hardcoded): B=2, T=2048 (512 frames x 4 animals), C=512, H=8 heads,
D=64. Block-causal mask = kron(tril(frames), ones(4,4)); key mask is all-ones
per the input spec (a numpy fallback handles the general case).

Sharding: core i handles batch b = i//4 and heads (2*(i%4), 2*(i%4)+1).
Wq/Wk/Wv are column-sharded (128 cols per core), Wp row-sharded (128 rows per
core). Each core emits a bf16 partial [T, C] output projection; the host sums
the 4 partials per batch (fp32) and adds bp.

On-core algorithm (bf16 matmul data, fp32 PSUM accumulation), fully
software-pipelined: per 512-token chunk t5, the q/k/v projections and V-block
transposes are emitted, then the attention groups for query chunks 2*t5 and
2*t5+1 flow through an S -> exp -> P@V pipeline with 2-group lookahead so the
PE never waits on the scalar-engine exp. Mask handled by a rank-64 indicator
matmul on diagonal blocks (-1e9 offside). The softmax denominator l rides as
row 64 of the accumulated O^T (ones column in V_aug); division by l happens
after the output projection, per 128-row tile, split between ACT and DVE.
"""

import math

import numpy as np
import ml_dtypes

import concourse.bass as bass
import concourse.bacc as bacc
import concourse.tile as tile
from concourse import mybir
from concourse.bass_utils import run_bass_kernel_spmd

B, T, C, H, D = 2, 2048, 512, 8, 64
NF, NA = 512, 4
NCORES = 8
HPC = 2            # heads per core
CPB = 4            # cores per batch
SCALE = 1.0 / math.sqrt(D)
NEG = -1.0e9
IC = 256           # query-chunk width
NQC = T // IC      # 8
GRP = 4            # j-blocks (128 keys each) per exp group

F32 = mybir.dt.float32
BF16 = mybir.dt.bfloat16
NPBF16 = ml_dtypes.bfloat16

USE_BIAS = False     # set by kernel() when any bias is nonzero
DBG_NOMASK = False


def _emit(ctx, tc, out_d, in_d):
    nc = tc.nc
    WDT = BF16
    ts = bass.ts
    Exp = mybir.ActivationFunctionType.Exp
    mult, add = mybir.AluOpType.mult, mybir.AluOpType.add

    const = ctx.enter_context(tc.tile_pool(name="const", bufs=1))
    big = ctx.enter_context(tc.tile_pool(name="big", bufs=1))

    # --- SBUF tiles -------------------------------------------------------
    sel = const.tile([65, 1], WDT)
    ident = const.tile([128, 128], WDT)
    ones_col = const.tile([128, 1], WDT)
    w_sb = {nm: const.tile([128, 512], WDT, tag=f"w_{nm}", name=f"w_{nm}")
            for nm in ("wq", "wk", "wv")}
    wps = [const.tile([64, 512], WDT, tag=f"wp{h}", name=f"wp{h}")
           for h in range(HPC)]
    if USE_BIAS:
        b3 = const.tile([1, 384], WDT, tag="b3")
        ones_row = const.tile([1, T], WDT, tag="ones")

    xT = big.tile([128, 4 * T], WDT, tag="xT")
    qA = [big.tile([128, T], WDT, tag=f"qA{h}", name=f"qA{h}") for h in range(HPC)]
    kA = [big.tile([128, T], WDT, tag=f"kA{h}", name=f"kA{h}") for h in range(HPC)]
    vT = big.tile([128, T], WDT, tag="vT")
    Vb = big.tile([128, 16 * 130], WDT, tag="Vb")
    OTs = [big.tile([65, T], WDT, tag=f"OT{h}", name=f"OT{h}") for h in range(HPC)]
    rl = big.tile([128, 2 * 16], F32, tag="rl")

    # --- input DMAs, split across the two HWDGE rings ---------------------
    # sync ring: wq, xt(t5=0), wk, wv, wp, xt(t5=2) in need order
    nc.sync.dma_start(out=w_sb["wq"][:], in_=in_d["wq"][:])
    for cb in range(4):
        nc.sync.dma_start(
            out=xT[:, cb * T : cb * T + 512], in_=in_d["xt"][ts(cb, 128), 0:512]
        )
    nc.sync.dma_start(out=w_sb["wk"][:], in_=in_d["wk"][:])
    nc.sync.dma_start(out=w_sb["wv"][:], in_=in_d["wv"][:])
    nc.sync.dma_start(out=wps[0][:], in_=in_d["wp"][0:64, :])
    nc.sync.dma_start(out=wps[1][:], in_=in_d["wp"][64:128, :])
    for cb in range(4):
        nc.sync.dma_start(
            out=xT[:, cb * T + 1024 : cb * T + 1536],
            in_=in_d["xt"][ts(cb, 128), 1024:1536],
        )
    # scalar ring: mask seeds first (needed by the first DiagE), then consts,
    # then xt(t5=1), then xt(t5=3). Masks are periodic with period 256; ship
    # one period and replicate in SBUF by doubling copies (DVE/GpSimd idle).
    nc.scalar.dma_start(out=kA[0][64:128, 0:256], in_=in_d["mask_k"][:])
    nc.scalar.dma_start(out=kA[1][0:64, 0:256], in_=in_d["mask_k"][:])
    nc.scalar.dma_start(out=qA[0][64:128, 0:256], in_=in_d["mask_q"][:])
    nc.scalar.dma_start(out=qA[1][0:64, 0:256], in_=in_d["mask_q"][:])
    nc.scalar.dma_start(out=ident[:], in_=in_d["ident"][:])
    nc.scalar.dma_start(out=sel[:], in_=in_d["sel65"][:])
    nc.scalar.dma_start(out=ones_col[:], in_=in_d["ones_col"][:])
    if USE_BIAS:
        nc.scalar.dma_start(out=b3[:], in_=in_d["b3"][:])
        nc.scalar.dma_start(out=ones_row[:], in_=in_d["ones_row"][:])
    for w in (256, 512, 1024):
        nc.vector.tensor_copy(kA[0][64:128, w : 2 * w], kA[0][64:128, 0:w])
        nc.vector.tensor_copy(kA[1][0:64, w : 2 * w], kA[1][0:64, 0:w])
        nc.gpsimd.tensor_copy(qA[0][64:128, w : 2 * w], qA[0][64:128, 0:w])
        nc.gpsimd.tensor_copy(qA[1][0:64, w : 2 * w], qA[1][0:64, 0:w])
    for cb in range(4):
        nc.scalar.dma_start(
            out=xT[:, cb * T + 512 : cb * T + 1024],
            in_=in_d["xt"][ts(cb, 128), 512:1024],
        )
    for cb in range(4):
        nc.scalar.dma_start(
            out=xT[:, cb * T + 1536 : cb * T + 2048],
            in_=in_d["xt"][ts(cb, 128), 1536:2048],
        )

    # Vb ones columns (constant): cols jb*130+64 and jb*130+129.
    for jb in range(16):
        nc.gpsimd.tensor_copy(Vb[:, jb * 130 + 64 : jb * 130 + 65], ones_col[:])
        nc.gpsimd.tensor_copy(Vb[:, jb * 130 + 129 : jb * 130 + 130], ones_col[:])

    # --- attention stage machinery ---------------------------------------
    # Per t5-pair (query chunks qcE=2*t5, qcO=2*t5+1):
    #   Pair stages: off-diagonal key blocks jb < 4*t5 are scored for BOTH
    #     query chunks in one 512-wide matmul; exp lands in a persistent
    #     SBUF tile pttP, consumed now for qcE and later for qcO.
    #   DiagE: qcE's two diagonal blocks (K=128 with mask rows), then tail.
    #   FreshO: qcO's four remaining blocks + deferred paired P@V, then tail.
    pss = ctx.enter_context(tc.tile_pool(name="pss", bufs=2, space="PSUM"))
    psot = ctx.enter_context(tc.tile_pool(name="psot", bufs=1, space="PSUM"))
    prl = ctx.enter_context(tc.tile_pool(name="prl", bufs=2, space="PSUM"))
    aux = ctx.enter_context(tc.tile_pool(name="aux", bufs=1, space="PSUM"))
    ptp = ctx.enter_context(tc.tile_pool(name="ptp", bufs=3))
    pttp = ctx.enter_context(tc.tile_pool(name="pttp", bufs=1))
    comb = ctx.enter_context(tc.tile_pool(name="comb", bufs=2))

    state = {}

    def hsl(h):
        return slice(0, 64) if h == 0 else slice(64, 128)

    def pv_mm(ot, h, jb, mov, start, stop):
        vbase = jb * 130 + h * 65
        nc.tensor.matmul(
            ot[:, h * IC : (h + 1) * IC], Vb[:, vbase : vbase + 65], mov,
            start=start, stop=stop,
        )

    def get_ot(key):
        if key not in state:
            state[key] = psot.tile([65, HPC * IC], F32, tag="ot", name=f"ot{key}")
        return state[key]

    def emit_proj(nm, dst, t5):
        ps = aux.tile([128, 512], F32, tag="aux", name=f"pj_{nm}{t5}")
        for cb in range(4):
            nc.tensor.matmul(
                ps[:],
                w_sb[nm][:, ts(cb, 128)],
                xT[:, cb * T + t5 * 512 : cb * T + (t5 + 1) * 512],
                start=(cb == 0),
                stop=(False if USE_BIAS else cb == 3),
            )
        if USE_BIAS:
            boff = {"wq": 0, "wk": 128, "wv": 256}[nm]
            nc.tensor.matmul(
                ps[:], b3[:, boff : boff + 128], ones_row[:, ts(t5, 512)],
                start=False, stop=True,
            )
        if nm == "wv":
            nc.vector.tensor_copy(dst[:, ts(t5, 512)], ps[:])
        else:
            nc.vector.tensor_copy(dst[0][0:64, ts(t5, 512)], ps[0:64, :])
            nc.vector.tensor_copy(dst[1][64:128, ts(t5, 512)], ps[64:128, :])

    def emit_vb(t5):
        for jb in range(4 * t5, 4 * t5 + 4):
            pv = aux.tile([128, 128], WDT, tag="aux", name=f"pv{jb}")
            nc.tensor.transpose(pv[:], vT[:, ts(jb, 128)], ident[:])
            nc.vector.tensor_copy(Vb[:, jb * 130 : jb * 130 + 64], pv[:, 0:64])
            nc.vector.tensor_copy(
                Vb[:, jb * 130 + 65 : jb * 130 + 129], pv[:, 64:128]
            )

    def emit_tail(qc):
        for t2 in (2 * qc, 2 * qc + 1):
            pr1 = prl.tile([128, 512], F32, tag="pr", name=f"pr1_{t2}")
            for h in range(HPC):
                nc.tensor.matmul(
                    pr1[:, h : h + 1], OTs[h][:, ts(t2, 128)], sel[:],
                    start=True, stop=True,
                )
            nc.vector.reciprocal(rl[:, 2 * t2 : 2 * t2 + 2], pr1[:, 0:2])
            pr0 = prl.tile([128, 512], F32, tag="pr", name=f"pr0_{t2}")
            nc.tensor.matmul(
                pr0[:], OTs[0][0:64, ts(t2, 128)], wps[0][:],
                start=True, stop=True,
            )
            nc.tensor.matmul(
                pr1[:], OTs[1][0:64, ts(t2, 128)], wps[1][:],
                start=True, stop=True,
            )
            tmp = comb.tile([128, 512], F32, tag="tmp")
            nc.vector.tensor_scalar_mul(tmp[:], pr0[:], rl[:, 2 * t2 : 2 * t2 + 1])
            ob = comb.tile([128, 512], WDT, tag="ob")
            nc.vector.scalar_tensor_tensor(
                ob[:], pr1[:], rl[:, 2 * t2 + 1 : 2 * t2 + 2], tmp[:], mult, add
            )
            nc.sync.dma_start(out=out_d[ts(t2, 128), :], in_=ob[:])

    class Pair:
        def __init__(self, t5, h, c0):
            self.t5, self.h, self.c0 = t5, h, c0

        def s_phase(self, idx):
            t5, h, c0 = self.t5, self.h, self.c0
            ps = pss.tile([128, 1024], F32, tag="ps", name=f"ps{idx}")
            self.ps = ps
            for k in range(2):
                jb = c0 + k
                nc.tensor.matmul(
                    ps[:, ts(k, 512)],
                    kA[h][hsl(h), ts(jb, 128)],
                    qA[h][hsl(h), ts(t5, 512)],
                    start=True, stop=True,
                )

        def c_phase(self):
            t5, h, c0 = self.t5, self.h, self.c0
            pkey = ("pttp", t5, h)
            if pkey not in state:
                state[pkey] = pttp.tile(
                    [128, 12 * 512], WDT, tag=f"pttP{h}", name=f"pttP{h}_{t5}"
                )
            pt = state[pkey]
            nc.scalar.activation(pt[:, c0 * 512 : (c0 + 2) * 512], self.ps[:], Exp)
            ot = get_ot(("ot", t5, "e"))
            for k in range(2):
                jb = c0 + k
                pv_mm(ot, h, jb, pt[:, jb * 512 : jb * 512 + IC],
                      start=(jb == 0), stop=False)

    class DiagE:
        def __init__(self, t5, h):
            self.t5, self.h = t5, h

        def s_phase(self, idx):
            t5, h = self.t5, self.h
            ps = pss.tile([128, 1024], F32, tag="ps", name=f"ps{idx}")
            self.ps = ps
            for k in range(2):
                jb = 4 * t5 + k
                nc.tensor.matmul(
                    ps[:, ts(k, IC)],
                    kA[h][:, ts(jb, 128)],
                    qA[h][:, ts(2 * t5, IC)],
                    start=True, stop=True,
                )

        def c_phase(self):
            t5, h = self.t5, self.h
            ptt = ptp.tile([128, 1024], WDT, tag="ptt", name="ptt")
            nc.scalar.activation(ptt[:, 0:512], self.ps[:, 0:512], Exp)
            ot = get_ot(("ot", t5, "e"))
            for k in range(2):
                jb = 4 * t5 + k
                pv_mm(ot, h, jb, ptt[:, ts(k, IC)],
                      start=(jb == 0), stop=(k == 1))
            nc.vector.tensor_copy(
                OTs[h][:, ts(2 * t5, IC)], ot[:, h * IC : (h + 1) * IC]
            )
            if h == HPC - 1:
                emit_tail(2 * t5)

    class FreshO:
        def __init__(self, t5, h):
            self.t5, self.h = t5, h

        def s_phase(self, idx):
            t5, h = self.t5, self.h
            ps = pss.tile([128, 1024], F32, tag="ps", name=f"ps{idx}")
            self.ps = ps
            qsl = qA[h][:, 2 * t5 * IC + IC : 2 * t5 * IC + 2 * IC]
            qso = qA[h][hsl(h), 2 * t5 * IC + IC : 2 * t5 * IC + 2 * IC]
            for k in range(4):
                jb = 4 * t5 + k
                if k < 2:
                    nc.tensor.matmul(
                        self.ps[:, ts(k, IC)],
                        kA[h][hsl(h), ts(jb, 128)], qso,
                        start=True, stop=True,
                    )
                else:
                    nc.tensor.matmul(
                        self.ps[:, ts(k, IC)],
                        kA[h][:, ts(jb, 128)], qsl,
                        start=True, stop=True,
                    )

        def c_phase(self):
            t5, h = self.t5, self.h
            ptt = ptp.tile([128, 1024], WDT, tag="ptt", name="ptt")
            nc.scalar.activation(ptt[:], self.ps[:], Exp)
            ot = get_ot(("ot", t5, "o"))
            pt = state.get(("pttp", t5, h))
            for jb in range(4 * t5):
                pv_mm(ot, h, jb, pt[:, jb * 512 + IC : (jb + 1) * 512],
                      start=(jb == 0), stop=False)
            for k in range(4):
                jb = 4 * t5 + k
                pv_mm(ot, h, jb, ptt[:, ts(k, IC)],
                      start=(jb == 0), stop=(k == 3))
            nc.vector.tensor_copy(
                OTs[h][:, 2 * t5 * IC + IC : 2 * t5 * IC + 2 * IC],
                ot[:, h * IC : (h + 1) * IC],
            )
            if h == HPC - 1:
                emit_tail(2 * t5 + 1)

    # --- main pipeline ----------------------------------------------------
    sidx = [0]
    for t5 in range(4):
        stages = []
        for h in range(HPC):
            for c0 in range(0, 4 * t5, 2):
                stages.append(Pair(t5, h, c0))
            stages.append(DiagE(t5, h))
        for h in range(HPC):
            stages.append(FreshO(t5, h))

        emit_proj("wq", qA, t5)
        emit_proj("wk", kA, t5)
        stages[0].s_phase(sidx[0]); sidx[0] += 1
        if len(stages) > 1:
            stages[1].s_phase(sidx[0]); sidx[0] += 1
        emit_proj("wv", vT, t5)
        emit_vb(t5)
        for i, st in enumerate(stages):
            st.c_phase()
            if i + 2 < len(stages):
                stages[i + 2].s_phase(sidx[0]); sidx[0] += 1


_PROGRAM_CACHE = {}
TRACE = False
_LAST = {}


def _build_program():
    key = ("prog", GRP, USE_BIAS, DBG_NOMASK)
    if key in _PROGRAM_CACHE:
        return _PROGRAM_CACHE[key]
    from contextlib import ExitStack

    nc = bacc.Bacc(trn_type="TRN2", target_bir_lowering=False, debug=False,
                   num_devices=NCORES)
    WDT = BF16
    in_d = {
        "xt": nc.dram_tensor("xt", [C, T], WDT, kind="ExternalInput").ap(),
        "wq": nc.dram_tensor("wq", [128, 512], WDT, kind="ExternalInput").ap(),
        "wk": nc.dram_tensor("wk", [128, 512], WDT, kind="ExternalInput").ap(),
        "wv": nc.dram_tensor("wv", [128, 512], WDT, kind="ExternalInput").ap(),
        "wp": nc.dram_tensor("wp", [128, C], WDT, kind="ExternalInput").ap(),
        "ident": nc.dram_tensor("ident", [128, 128], WDT, kind="ExternalInput").ap(),
        "mask_k": nc.dram_tensor("mask_k", [64, 256], WDT, kind="ExternalInput").ap(),
        "mask_q": nc.dram_tensor("mask_q", [64, 256], WDT, kind="ExternalInput").ap(),
        "sel65": nc.dram_tensor("sel65", [65, 1], WDT, kind="ExternalInput").ap(),
        "ones_col": nc.dram_tensor("ones_col", [128, 1], WDT,
                                   kind="ExternalInput").ap(),
    }
    if USE_BIAS:
        in_d["b3"] = nc.dram_tensor("b3", [1, 384], WDT, kind="ExternalInput").ap()
        in_d["ones_row"] = nc.dram_tensor(
            "ones_row", [1, T], WDT, kind="ExternalInput").ap()
    out_d = nc.dram_tensor("out", [T, C], BF16, kind="ExternalOutput").ap()
    with tile.TileContext(nc) as tc:
        with ExitStack() as ctx:
            _emit(ctx, tc, out_d, in_d)
    nc.compile()
    _PROGRAM_CACHE[key] = nc
    return nc


def _consts():
    f = np.float32
    rr = np.arange(64)
    jj = np.arange(T)
    # mask_k[r, j] = 1 where r == 32*(jb%2) + (j%128)//4 (jb = j//128)
    mk = (rr[:, None] == 32 * ((jj[None, :] // 128) % 2) + (jj[None, :] % 128) // NA)
    mask_k = mk.astype(f)
    # mask_q[r, i] = NEG where (i%256)//4 < r
    mask_q = np.where((jj[None, :] % IC) // NA < rr[:, None], f(NEG), f(0.0)).astype(f)
    sel = np.zeros((65, 1), dtype=f)
    sel[64, 0] = 1.0
    ident = np.eye(128, dtype=f)
    return mask_k, mask_q, sel, ident


def _sbuf_w(w):
    # DRAM [512, 128] -> SBUF-layout [128, 512]: w_sb[p, cb*128+f] = W[cb*128+p, f]
    return np.ascontiguousarray(
        w.reshape(4, 128, 128).transpose(1, 0, 2).reshape(128, 512)
    )


def _numpy_reference(x, mask, Wq, bq, Wk, bk, Wv, bv, Wp, bp):
    b, t, c = x.shape
    h, d = H, c // H
    scale = 1.0 / math.sqrt(d)
    tril = np.tril(np.ones((NF, NF), dtype=np.float32))
    block = np.kron(tril, np.ones((NA, NA), dtype=np.float32))

    def heads(w, bias):
        return (x @ w + bias).reshape(b, t, h, d).transpose(0, 2, 1, 3)

    q, k, v = heads(Wq, bq), heads(Wk, bk), heads(Wv, bv)
    att = np.einsum("bhqd,bhkd->bhqk", q, k) * scale
    allowed = block[None, None] * mask[:, None, None, :].astype(np.float32)
    att = np.where(allowed == 0, -np.inf, att)
    att = att - att.max(axis=-1, keepdims=True)
    att = np.exp(att)
    att = att / att.sum(axis=-1, keepdims=True)
    y = np.einsum("bhqk,bhkd->bhqd", att, v)
    y = y.transpose(0, 2, 1, 3).reshape(b, t, c)
    return (y @ Wp + bp).astype(np.float32)


def kernel(**inputs):
    global USE_BIAS
    x = np.asarray(inputs["x"], dtype=np.float32)
    mask = np.asarray(inputs["mask"])
    Wq = np.asarray(inputs["Wq"], dtype=np.float32)
    bq = np.asarray(inputs["bq"], dtype=np.float32)
    Wk = np.asarray(inputs["Wk"], dtype=np.float32)
    bk = np.asarray(inputs["bk"], dtype=np.float32)
    Wv = np.asarray(inputs["Wv"], dtype=np.float32)
    bv = np.asarray(inputs["bv"], dtype=np.float32)
    Wp = np.asarray(inputs["Wp"], dtype=np.float32)
    bp = np.asarray(inputs["bp"], dtype=np.float32)

    if not np.all(np.asarray(mask) == 1):
        return _numpy_reference(x, mask, Wq, bq, Wk, bk, Wv, bv, Wp, bp)

    USE_BIAS = bool(np.any(bq) or np.any(bk) or np.any(bv))
    nc = _build_program()
    mask_k, mask_q, sel, ident = _consts()
    bf = NPBF16
    in_maps = []
    for core in range(NCORES):
        b = core // CPB
        hp = core % CPB
        cs = slice(hp * 128, (hp + 1) * 128)
        im = {
            "xt": np.ascontiguousarray(x[b].T).astype(bf),
            "wq": (_sbuf_w(np.ascontiguousarray(Wq[:, cs]))
                   * np.float32(SCALE)).astype(bf),
            "wk": _sbuf_w(np.ascontiguousarray(Wk[:, cs])).astype(bf),
            "wv": _sbuf_w(np.ascontiguousarray(Wv[:, cs])).astype(bf),
            "wp": np.ascontiguousarray(Wp[cs, :]).astype(bf),
            "ident": ident.astype(bf),
            "mask_k": np.ascontiguousarray(mask_k[:, 0:256]).astype(bf),
            "mask_q": np.ascontiguousarray(mask_q[:, 0:256]).astype(bf),
            "sel65": sel.astype(bf),
            "ones_col": np.ones((128, 1), dtype=bf),
        }
        if USE_BIAS:
            b3 = np.concatenate([bq[cs] * np.float32(SCALE), bk[cs], bv[cs]])
            im["b3"] = b3.reshape(1, 384).astype(bf)
            im["ones_row"] = np.ones((1, T), dtype=bf)
        in_maps.append(im)
    rr = run_bass_kernel_spmd(
        nc, in_maps, list(range(NCORES)), trace=TRACE,
        tmpdir=_LAST.get("tmpdir"),
    )
    _LAST["results"] = rr
    res = rr.results
    out = np.zeros((B, T, C), dtype=np.float32)
    for b in range(B):
        acc = res[b * CPB]["out"].astype(np.float32)
        for j in range(1, CPB):
            acc = acc + res[b * CPB + j]["out"].astype(np.float32)
        out[b] = acc + bp[None, :]
    return out
